# Initial kernel scaffold
#
"""Trainium2 Bass kernel for a 4-layer spiking actor network (SNN scan).

Reference computation (per timestep t of 50, batch B=2048):
    for layer i in 0..3:
        c_i = 0.5*c_i + in @ W_i.T + b_i
        v_i = 0.75*v_i*(1-s_i) + c_i
        s_i = (v_i > 0.5)
        in = s_i
    out = sum_t s_3 / 50

Strategy:
  - Data parallel over 8 NeuronCores: batch 2048 -> 256 per core; weights
    replicated. No cross-core communication.
  - On-chip layout: activations/state as [feature(partitions), batch(free)].
    A layer's 512 features are 4 chunks of 128 partitions, packed along the
    free dim of one [128, 1024] tile; free-slice k of that tile is exactly
    k-tile k of the next layer's contraction.
  - Matmuls in float32r (hardware rounds inputs to 11 mantissa bits) with a
    hi/lo split: W = rne11(W) + rne11(W - rne11(W)), x likewise. Spikes are
    0/1 (exact). Result is fp32-class precision at 1 cycle/column (4x the
    plain-fp32 matmul rate). Dropping any lo term fails the 2e-2 gate (the
    spiking dynamics amplify perturbations ~1000x; measured 2.7-5e-2).
  - Membrane current c lives permanently in PSUM; the tensor engine
    accumulates W@s on top of it (start=False), ACT does the in-place decay
    c <- 0.5*c + b between steps. Layer 3 (fo=32) packs its hi|lo weight
    copies along the lhsT free dim -> one M=64 matmul per k-tile; the two
    current halves superpose linearly and are summed when v is formed.
  - Elementwise engine split (the measured 3.9x win of this kernel): v-add /
    spike-compare / w34-mult on DVE, n75 = 0.75*(1-s) as a LINEAR activation
    from s on ACT (compare-op tensor_scalar is a slow path), decay on ACT,
    output accumulation on DVE. GPSIMD gets NOTHING: it shares an SBUF port
    pair with DVE's 2-port tensor_scalar modes and whoever issues first
    fully blocks the other (measured 7-16us per op under contention).
  - The (timestep, layer) grid is emitted as a layer-pipelined wavefront
    (wave w = layer i at t = w - i): the four layer-steps of a wave are
    mutually independent, so PE/ACT/DVE/DMA all stay busy. Emitting t-major
    instead serializes the engines through the recurrence chain (measured
    ~3x slower).
  - DMA emission order: x[0], layer-0 weights, x[1], then remaining layers
    in wave order, so the PE starts ~35us earlier. Dummy dep-free matmuls
    (WARM) fill the startup DMA wait and drain waves to hold the HAM clock
    gate at 2.4 GHz.
  - Measured on core 0: 3.10 ms (session start) -> 0.738 ms. PE ~96% busy at
    ~121 ns per LDW+MM pair (N=256 streaming floor is 107 ns); remaining
    span is ~619 us matmul streaming + ~85 us NX/LDW overhead + ramp/drain.
"""
import os
import sys
import numpy as np
from contextlib import ExitStack

if os.path.isdir("/opt/trn_rl_repo"):
    sys.path.insert(0, "/opt/trn_rl_repo")

import concourse.bass as bass
import concourse.tile as tile
from concourse import bacc, mybir
from concourse import bass_utils

F32 = mybir.dt.float32
F32R = mybir.dt.float32r

B, S, T, H, A = 2048, 512, 50, 512, 32
NCORES = 8
BL = B // NCORES  # 256 batch per core
NK = 4            # k-tiles per 512-dim contraction
NM = 4            # output chunks per 512-dim layer
CDECAY, VDECAY, VTH = 0.5, 0.75, 0.5


def rne11(x: np.ndarray) -> np.ndarray:
    """Round fp32 to 11 explicit mantissa bits, nearest-even (== HW float32r)."""
    u = x.view(np.uint32)
    lsb = (u >> np.uint32(12)) & np.uint32(1)
    u2 = (u + np.uint32(0x7FF) + lsb) & np.uint32(0xFFFFF000)
    return u2.view(np.float32)


def split_f32r(x: np.ndarray):
    hi = rne11(np.ascontiguousarray(x, dtype=np.float32))
    lo = rne11((x - hi).astype(np.float32))
    return hi, lo


ABL = int(os.environ.get("ABL", "0"))  # timing ablations: 1=no decay, 2=no n75/w34
EARLY_DECAY = int(os.environ.get("EARLY_DECAY", "1"))
WARM = int(os.environ.get("WARM", "1"))  # HAM warm-keeper dummy matmuls
# merge layer-0's two hi-weight matmuls (W_hi@x_hi + W_hi@x_lo) into one
# N=512 matmul whose output AP aliases both 256-col halves onto the same
# PSUM region (stride-0 dim; per-element has_written bits accumulate).
# Measured: numerically exact on HW but SLOWER (874us vs 738us) -- the
# non-contiguous PSUM out AP slows the matmul drain. Keep off.
XMERGE = int(os.environ.get("XMERGE", "0"))


def build_nc(T_steps: int):
    nc = bacc.Bacc("TRN2", target_bir_lowering=False, debug=False,
                   num_devices=NCORES)

    dims_in = [S, H, H, H]
    dims_out = [H, H, H, A]

    # DRAM tensors. x ships pre-packed as [T, 128, NK*2*BL]: partition p,
    # k-tile block k holds [x_hi_k (BL cols) | x_lo_k (BL cols)] -- so the
    # DMA per step is one contiguous [128, NK*2*BL] copy, block k is k-tile
    # k of the layer-0 contraction, and the hi|lo halves are adjacent so a
    # single N=2*BL matmul can stream both (XMERGE).
    xc_d = nc.dram_tensor("xc", [T_steps, 128, NK * 2 * BL], F32R,
                          kind="ExternalInput")
    wr_d, wl_d, b_d = [], [], []
    for i in range(4):
        wr_d.append(nc.dram_tensor(f"w{i}r", [dims_in[i], dims_out[i]], F32R,
                                   kind="ExternalInput"))
        wl_d.append(nc.dram_tensor(f"w{i}l", [dims_in[i], dims_out[i]], F32R,
                                   kind="ExternalInput"))
        b_d.append(nc.dram_tensor(f"b{i}", [dims_out[i], 1], F32,
                                  kind="ExternalInput"))
    out_d = nc.dram_tensor("out", [A, BL], F32, kind="ExternalOutput")

    with tile.TileContext(nc) as tc, ExitStack() as ctx:
        wpool = ctx.enter_context(tc.tile_pool(name="weights", bufs=1))
        spool = ctx.enter_context(tc.tile_pool(name="state", bufs=1))
        xpool = ctx.enter_context(tc.tile_pool(name="xin", bufs=4))
        vpool = ctx.enter_context(tc.tile_pool(name="vws", bufs=2))
        opool = ctx.enter_context(tc.tile_pool(name="outs", bufs=1))
        ppool = ctx.enter_context(tc.tile_pool(name="psum", bufs=1,
                                               space="PSUM"))

        # ---- x staging (defined early; first DMAs emitted below) ----
        x_stage = {}

        def stage_x(t):
            xc_t = xpool.tile([128, NK * 2 * BL], F32R, name="xc", tag="xc")
            nc.sync.dma_start(out=xc_t[:], in_=xc_d.ap()[t])
            x_stage[t] = xc_t

        # ---- load weights/biases (persistent). DMA queue order == emission
        # order, so emit x[0] first, then per-layer weights in wave order:
        # wave w first needs layer w's weights, so the PE can start ~35us
        # earlier than if all 8.4MB of weights had to land first.
        stage_x(0)
        wr_t = [[None] * NK for _ in range(4)]
        wl_t = [[None] * NK for _ in range(4)]
        b_t = [None] * 4
        for i in range(4):
            fo = dims_out[i]
            for k in range(NK):
                if i == 3:
                    # layer 3 (fo=32): pack hi|lo weight copies along the lhsT
                    # free dim -> one M=64 matmul per k-tile instead of two
                    # M=32 ones. The hi and lo current recurrences superpose
                    # linearly; halves are summed when v is formed.
                    w3c = wpool.tile([128, 2 * A], F32R, name=f"w3c{k}",
                                     tag=f"w3c{k}")
                    nc.sync.dma_start(out=w3c[:, 0:A],
                                      in_=wr_d[i].ap()[k * 128:(k + 1) * 128, :])
                    nc.sync.dma_start(out=w3c[:, A:2 * A],
                                      in_=wl_d[i].ap()[k * 128:(k + 1) * 128, :])
                    wr_t[i][k] = w3c
                    continue
                wr_t[i][k] = wpool.tile([128, fo], F32R, name=f"w{i}r{k}", tag=f"w{i}r{k}")
                nc.sync.dma_start(out=wr_t[i][k][:],
                                  in_=wr_d[i].ap()[k * 128:(k + 1) * 128, :])
                wl_t[i][k] = wpool.tile([128, fo], F32R, name=f"w{i}l{k}", tag=f"w{i}l{k}")
                nc.sync.dma_start(out=wl_t[i][k][:],
                                  in_=wl_d[i].ap()[k * 128:(k + 1) * 128, :])
            nchunk = fo // 128 if fo >= 128 else 1
            if i == 3:
                # bias over both halves: [b3 ; 0]
                b_t[i] = wpool.tile([2 * A, 1], F32, name="b3", tag="b3")
                nc.vector.memset(b_t[i][:], 0.0)
                nc.sync.dma_start(out=b_t[i][0:A, :], in_=b_d[i].ap()[0:A, :])
                continue
            b_t[i] = wpool.tile([128, nchunk], F32, name=f"b{i}", tag=f"b{i}")
            for m in range(nchunk):
                pp = min(128, fo)
                nc.sync.dma_start(out=b_t[i][0:pp, m:m + 1],
                                  in_=b_d[i].ap()[m * 128:m * 128 + pp, :])
            if i == 0:
                stage_x(1)
        # per-partition 0.75 constant, bias operand for the n75 activation
        b75_t = wpool.tile([128, 1], F32, name="b75", tag="b75")
        nc.vector.memset(b75_t[:], VDECAY)
        # dep-free dummy operand for HAM warm-keeper matmuls
        warm_t = wpool.tile([128, 128], F32, name="warm", tag="warm")
        nc.vector.memset(warm_t[:], 0.0)

        # ---- persistent PSUM state: c per layer + output accumulator ----
        c_ps = [
            ppool.tile([128, NM * BL], F32, name="c0", tag="c0"),
            ppool.tile([128, NM * BL], F32, name="c1", tag="c1"),
            ppool.tile([128, NM * BL], F32, name="c2", tag="c2"),
            ppool.tile([2 * A, BL], F32, name="c3", tag="c3"),
        ]
        # output spike accumulator lives in SBUF, accumulated by DVE
        out_acc = opool.tile([A, BL], F32, name="outacc", tag="outacc")
        nc.vector.memset(out_acc[:], 0.0)

        # ---- persistent SBUF state: w34 (= 0.75 * v * (v<=vth)) ----
        w34_t = [
            spool.tile([128, NM * BL], F32, name="w34_0", tag="w34_0"),
            spool.tile([128, NM * BL], F32, name="w34_1", tag="w34_1"),
            spool.tile([128, NM * BL], F32, name="w34_2", tag="w34_2"),
            spool.tile([A, BL], F32, name="w34_3", tag="w34_3"),
        ]
        for i in range(4):
            nc.vector.memset(w34_t[i][:], 0.0)

        Ident = mybir.ActivationFunctionType.Identity
        AOT = mybir.AluOpType

        # rotating spike tiles per layer (consumed by the next layer one
        # wave later)
        s_tiles = [None] * 4

        def emit_layer(i, t, layer_r):
            fo = dims_out[i]
            nchunk = fo // 128 if fo >= 128 else 1
            pp = min(128, fo)
            # layer 3 keeps separate hi/lo current halves on 2*A partitions
            dp = 2 * A if i == 3 else pp
            ps = c_ps[i]

            def emit_decay():
                # c <- 0.5*c + b (in place, per chunk for per-chunk bias)
                for m in range(nchunk):
                    nc.scalar.activation(
                        ps[0:dp, m * BL:(m + 1) * BL],
                        ps[0:dp, m * BL:(m + 1) * BL],
                        Ident, bias=b_t[i][0:dp, m:m + 1], scale=CDECAY)

            if t > 0 and ABL != 1 and not EARLY_DECAY:
                emit_decay()

            # accumulate W @ s (or W @ x hi/lo for layer 0)
            for m in range(nchunk):
                mm_idx = 0
                for k in range(NK):
                    out_ap = ps[0:dp, m * BL:(m + 1) * BL]
                    if i == 0:
                        # layer_r is the packed x tile: [x_hi_k | x_lo_k]
                        xk = layer_r[:, k * 2 * BL:(k + 1) * 2 * BL]
                        lhs_r = wr_t[i][k][:, m * pp:(m + 1) * pp]
                        lhs_l = wl_t[i][k][:, m * pp:(m + 1) * pp]
                        if XMERGE:
                            # one N=2*BL matmul streams both x halves; the
                            # output AP aliases them onto the same PSUM
                            # region (stride-0 dim -> has_written accumulate)
                            alias = out_ap.unsqueeze(1).to_broadcast(
                                [dp, 2, BL])
                            terms = [(lhs_r, xk, alias),
                                     (lhs_l, xk[:, 0:BL], out_ap)]
                        else:
                            terms = [(lhs_r, xk[:, 0:BL], out_ap),
                                     (lhs_r, xk[:, BL:2 * BL], out_ap),
                                     (lhs_l, xk[:, 0:BL], out_ap)]
                    else:
                        rhs_r = layer_r[:, k * BL:(k + 1) * BL]
                        if i == 3:
                            # merged hi|lo lhsT: one M=64 matmul per k-tile
                            terms = [(wr_t[i][k][:, 0:2 * A], rhs_r, out_ap)]
                        else:
                            lhs_r = wr_t[i][k][:, m * pp:(m + 1) * pp]
                            lhs_l = wl_t[i][k][:, m * pp:(m + 1) * pp]
                            terms = [(lhs_r, rhs_r, out_ap),
                                     (lhs_l, rhs_r, out_ap)]
                    for lhs, rhs, oap in terms:
                        # exactly one start=True per PSUM bank (at t=0)
                        first_in_bank = (t == 0 and mm_idx == 0
                                         and (m * BL) % 512 == 0)
                        nc.tensor.matmul(oap, lhs, rhs,
                                         start=first_in_bank, stop=True,
                                         skip_group_check=True)
                        mm_idx += 1

            if t == 0:
                # bias add after the first accumulation
                for m in range(nchunk):
                    nc.scalar.activation(
                        ps[0:dp, m * BL:(m + 1) * BL],
                        ps[0:dp, m * BL:(m + 1) * BL],
                        Ident, bias=b_t[i][0:dp, m:m + 1], scale=1.0)

            # v = c + w34_old   (w34 = 0.75 * v_prev * not-spiked)
            v_t = vpool.tile([pp, nchunk * BL], F32, name=f"v{i}", tag=f"v{i}")
            nc.vector.tensor_tensor(v_t[:], ps[0:pp, 0:nchunk * BL],
                                    w34_t[i][:], AOT.add)
            if i == 3:
                # fold in the lo-half current
                nc.vector.tensor_tensor(v_t[:], v_t[:], ps[A:2 * A, 0:BL],
                                        AOT.add)
            # spikes (0/1) in f32r, ready to be next layer's rhs
            s_t = vpool.tile([pp, nchunk * BL], F32R, name=f"s{i}",
                             tag=f"s{i}", bufs=3)
            nc.vector.tensor_scalar(s_t[:], v_t[:], VTH, None, AOT.is_gt)
            if EARLY_DECAY and ABL != 1 and t + 1 < T_steps:
                # psum has been read (v-add); decay it for step t+1 now.
                # Emitted BEFORE n75 on the scalar queue: the decay gates the
                # next wave's matmuls, n75 only gates w34.
                emit_decay()
            if ABL != 2:
                # n75 = 0.75 * (1 - s), computed linearly from s (s is exactly
                # 0/1). Avoids the compare-op slow path and keeps GPSIMD idle:
                # GPSIMD shares an SBUF port pair with DVE's 2-port perf modes,
                # so any GPSIMD op can fully block DVE tensor_scalar ops.
                n75_t = vpool.tile([pp, nchunk * BL], F32, name=f"n{i}",
                                   tag=f"n{i}")
                nc.scalar.activation(n75_t[:], s_t[:], Ident,
                                     bias=b75_t[0:pp, :], scale=-VDECAY)
                nc.vector.tensor_tensor(w34_t[i][:], v_t[:], n75_t[:],
                                        AOT.mult)
            s_tiles[i] = s_t

        def emit_warm(n, ps):
            # junk matmuls with no data deps: they run in PE-idle windows
            # (startup DMA wait, pipeline drain) and keep the HAM clock gate
            # at K=8/8 so the surrounding real matmuls run at 2.4 GHz. The
            # target region must be dead: c1 pre-loop (its first real matmul
            # is start=True at t=0), c0 during drain (last read at wave T-1).
            for _ in range(n):
                nc.tensor.matmul(ps[0:64, 0:64], warm_t[:, 0:64],
                                 warm_t[:, 64:128], start=True, stop=True,
                                 skip_group_check=True)

        if WARM:
            emit_warm(30, c_ps[1])

        # ---- wavefront over (timestep, layer): wave w runs layer i at
        # t = w - i; the four layer-steps in a wave are independent, so
        # every engine has work in every wave. x[0] and x[1] were staged
        # above, interleaved with the weight loads.
        for w in range(T_steps + 4):
            if WARM and w >= T_steps:
                # hold the PE clock warm through the drain waves
                emit_warm(10, c_ps[0])
            if w >= 1 and w + 1 < T_steps:
                # prefetch next wave's x one wave ahead of use
                stage_x(w + 1)
            prev_s = list(s_tiles)  # spikes produced in the previous wave
            for i in range(4):
                t = w - i
                if not (0 <= t < T_steps):
                    continue
                if i == 0:
                    layer_r = x_stage.pop(t)
                else:
                    layer_r = prev_s[i - 1]
                emit_layer(i, t, layer_r)
            t3 = w - 4
            if 0 <= t3 < T_steps:
                # accumulate output spikes on DVE (s3 is 0/1 in f32r)
                nc.vector.tensor_tensor(out_acc[:], out_acc[:],
                                        prev_s[3][:], AOT.add)

        # ---- drain output ----
        nc.sync.dma_start(out=out_d.ap(), in_=out_acc[:])

    nc.compile()
    return nc


def make_in_maps(x: np.ndarray, Ws, bs, T_steps: int):
    """Shard x over batch, split everything into f32r hi/lo pieces."""
    in_maps = []
    shared = {}
    for i in range(4):
        wt = np.ascontiguousarray(Ws[i].T)  # [fin, fout]
        hi, lo = split_f32r(wt)
        shared[f"w{i}r"] = hi
        shared[f"w{i}l"] = lo
        shared[f"b{i}"] = np.ascontiguousarray(
            bs[i].reshape(-1, 1).astype(np.float32))
    for c in range(NCORES):
        xb = np.ascontiguousarray(
            x[c * BL:(c + 1) * BL, :, :T_steps].transpose(2, 1, 0))
        # pack [T, S, BL] -> [T, 128, NK, BL] (see xc dram comment)
        xb = np.ascontiguousarray(
            xb.reshape(T_steps, NK, 128, BL).transpose(0, 2, 1, 3))
        hi, lo = split_f32r(xb)
        # interleave hi|lo per k-tile: [T, 128, NK, 2, BL]
        xcat = np.ascontiguousarray(
            np.stack([hi, lo], axis=3).reshape(T_steps, 128, NK * 2 * BL))
        m = dict(shared)
        m["xc"] = xcat
        in_maps.append(m)
    return in_maps


_NC_CACHE = {}


def run(x, Ws, bs, T_steps=T, trace=False):
    if T_steps not in _NC_CACHE:
        _NC_CACHE[T_steps] = build_nc(T_steps)
    nc = _NC_CACHE[T_steps]
    in_maps = make_in_maps(x, Ws, bs, T_steps)
    res = bass_utils.run_bass_kernel_spmd(
        nc, in_maps, core_ids=list(range(NCORES)), trace=trace)
    outs = []
    for c in range(NCORES):
        o = res.results[c]["out"]  # [A, BL] spike-count sums
        outs.append(o.T)  # [BL, A]
    full = np.concatenate(outs, axis=0) / np.float32(T_steps)
    return full.astype(np.float32), res


def kernel(**inputs) -> np.ndarray:
    x = np.asarray(inputs["x"], dtype=np.float32)
    Ws = [np.asarray(inputs[f"W{i}"], dtype=np.float32) for i in range(4)]
    bs = [np.asarray(inputs[f"b{i}"], dtype=np.float32) for i in range(4)]
    out, _ = run(x, Ws, bs, T_steps=x.shape[2])
    return out



# revision 4
# speedup vs baseline: 1.0778x; 1.0778x over previous
"""Trainium2 Bass kernel for a 4-layer spiking actor network (SNN scan).

Reference computation (per timestep t of 50, batch B=2048):
    for layer i in 0..3:
        c_i = 0.5*c_i + in @ W_i.T + b_i
        v_i = 0.75*v_i*(1-s_i) + c_i
        s_i = (v_i > 0.5)
        in = s_i
    out = sum_t s_3 / 50

Strategy (v2):
  - Data parallel over 8 NeuronCores: batch 2048 -> 256 per core; weights
    replicated. No cross-core communication.
  - Matmul precision scheme: W = W_hi(f32r,12b) + W_lo ladder. The hi terms
    run as f32r matmuls (1 cyc/col). The correction terms run as fp8-e5m2
    DoubleRow matmuls (2 k-tiles per instruction, 0.5 cyc/col) using scaled
    two-term e5m2 ladders; power-of-2 scales split across lhsT/rhs keep
    every operand inside e5m2's normal range.
  - Spikes live in ONE fp8e5 tile valued s*2^-8 (exact); all spike-side
    lhsT weights are prescaled by 2^8 (exact in f32r / absorbed into the
    e5m2 ladder scale), so hi and lo matmuls share the same rhs and no
    separate f32r spike tile or cast op exists.
  - Elementwise chain per layer uses scalar_tensor_tensor fusion:
        v  = VS_old*0.75 + c          (DVE stt, reads PSUM)
        s8 = (v > 0.5)*2^-8 -> fp8    (DVE tensor_scalar fused)
        VS = (v <= 0.5)*v             (DVE stt, in0==in1)
    replacing the v-add/compare/n75/w34 4-op chain (n75 ACT op gone).
    Decay c = 0.5c + b stays on ACT (per-chunk bias).
  - The (timestep, layer) grid is a layer-pipelined wavefront as before.
"""
import os
import sys
import numpy as np
from contextlib import ExitStack

if os.path.isdir("/opt/trn_rl_repo"):
    sys.path.insert(0, "/opt/trn_rl_repo")

import ml_dtypes
import concourse.bass as bass
import concourse.tile as tile
from concourse import bacc, mybir
from concourse import bass_utils

F32 = mybir.dt.float32
F32R = mybir.dt.float32r
F8E5 = mybir.dt.float8e5
E5 = ml_dtypes.float8_e5m2
DRMODE = mybir.MatmulPerfMode.DoubleRow

B, S, T, H, A = 2048, 512, 50, 512, 32
NCORES = 8
BL = B // NCORES  # 256 batch per core
NK = 4            # k-tiles per 512-dim contraction
NM = 4            # output chunks per 512-dim layer
CDECAY, VDECAY, VTH = 0.5, 0.75, 0.5

# ---- precision / scheduling flags ----
T2MODE = os.environ.get("T2MODE", "dr3")    # W_hi @ x_lo: f32r|dr1|dr2|dr3
T3MODE = os.environ.get("T3MODE", "dr3")    # W_lo @ x   : f32r|dr3
LO12MODE = os.environ.get("LO12MODE", "dr2")  # L1/L2 W_lo @ s: f32r|dr1|dr2
SFP8 = int(os.environ.get("SFP8", "0"))     # spikes as fp8e5 (s*2^-8);
# requires mixed f32r-lhsT x fp8-rhs matmuls, which the walrus verifier
# rejects (NCC_IBIR034) -- keep 0.
WARM = int(os.environ.get("WARM", "1"))
EARLY_DECAY = int(os.environ.get("EARLY_DECAY", "1"))

assert T2MODE in ("dr1", "dr2", "dr3"), "f32r T2 needs the old x packing"
assert T3MODE in ("f32r", "dr3")
assert LO12MODE in ("f32r", "dr1", "dr2")
assert not (SFP8 and LO12MODE == "f32r"), "f32r lo needs an f32r spike tile"

SS = 256.0 if SFP8 else 1.0   # spike-side lhsT prescale (2^8)
SSC = 1.0 / 256.0             # spike fp8 value scale (2^-8)
# e5m2 ladder scales
T2WS, T2XS = 2.0**-4, 2.0**4      # W_hi*2^-4 @ x_lo*2^4
T3WS, T3XS = 2.0**8, 2.0**-8      # W_lo*2^8 @ x*2^-8
LOWS = 2.0**8                     # L1/L2 W_lo*2^8 @ s*2^-8 (needs SFP8)


def rne11(x: np.ndarray) -> np.ndarray:
    """Round fp32 to 11 explicit mantissa bits, nearest-even (== HW float32r)."""
    u = np.ascontiguousarray(x, dtype=np.float32).view(np.uint32)
    lsb = (u >> np.uint32(12)) & np.uint32(1)
    u2 = (u + np.uint32(0x7FF) + lsb) & np.uint32(0xFFFFF000)
    return u2.view(np.float32)


def split_f32r(x: np.ndarray):
    hi = rne11(np.ascontiguousarray(x, dtype=np.float32))
    lo = rne11((x - hi).astype(np.float32))
    return hi, lo


def q8(x, scale):
    """e5m2 bits of x*scale."""
    return (np.asarray(x, np.float32) * np.float32(scale)).astype(E5)


def q8v(x, scale):
    """fp32 value represented by e5m2(x*scale), descaled."""
    return q8(x, scale).astype(np.float32) * np.float32(1.0 / scale)


def pack_dr(w8: np.ndarray) -> np.ndarray:
    """[fin, fout] e5m2 -> [2 pairs, 128, 2*fout] DoubleRow lhsT layout."""
    fin, fout = w8.shape
    assert fin == 512
    return np.ascontiguousarray(
        w8.reshape(2, 2, 128, fout).transpose(0, 2, 1, 3).reshape(2, 128, 2 * fout))


# number of fp8 x pieces shipped per step: [xl8a, xl8b?, x8a?, x8b?]
def x_pieces():
    p = []
    if T2MODE.startswith("dr"):
        p.append("xl8a")
        if T2MODE == "dr3":
            p.append("xl8b")
    if T3MODE == "dr3":
        p += ["x8a", "x8b"]
    return p


def build_nc(T_steps: int):
    nc = bacc.Bacc("TRN2", target_bir_lowering=False, debug=False,
                   num_devices=NCORES)

    dims_out = [H, H, H, A]
    pieces = x_pieces()
    NP = len(pieces)

    xhi_d = nc.dram_tensor("xhi", [T_steps, 128, NK * BL], F32R,
                           kind="ExternalInput")
    x8_d = None
    if NP:
        x8_d = nc.dram_tensor("x8", [T_steps, 128, NP * NK * BL], F8E5,
                              kind="ExternalInput")

    # f32r hi weights (+ f32r lo where configured)
    wr_d, wl_d, b_d = [], [None] * 4, []
    for i in range(4):
        wr_d.append(nc.dram_tensor(f"w{i}r", [S, dims_out[i]], F32R,
                                   kind="ExternalInput"))
        b_d.append(nc.dram_tensor(f"b{i}", [dims_out[i], 1], F32,
                                  kind="ExternalInput"))
    if T3MODE == "f32r":
        wl_d[0] = nc.dram_tensor("w0l", [S, H], F32R, kind="ExternalInput")
    if LO12MODE == "f32r":
        for i in (1, 2):
            wl_d[i] = nc.dram_tensor(f"w{i}l", [S, H], F32R,
                                     kind="ExternalInput")
    wl_d[3] = nc.dram_tensor("w3l", [S, A], F32R, kind="ExternalInput")

    # fp8 DR weight tensors: dict name -> dram
    dr_d = {}

    def dr_tensor(name, fout):
        dr_d[name] = nc.dram_tensor(name, [2, 128, 2 * fout], F8E5,
                                    kind="ExternalInput")

    if T2MODE.startswith("dr"):
        dr_tensor("wh8a", H)
        if T2MODE in ("dr2", "dr3"):
            dr_tensor("wh8b", H)
    if T3MODE == "dr3":
        dr_tensor("wl0a", H)
        dr_tensor("wl0b", H)
    if LO12MODE.startswith("dr"):
        for i in (1, 2):
            dr_tensor(f"wl{i}a", H)
            if LO12MODE == "dr2":
                dr_tensor(f"wl{i}b", H)

    out_d = nc.dram_tensor("out", [A, BL], F32, kind="ExternalOutput")

    with tile.TileContext(nc) as tc, ExitStack() as ctx:
        wpool = ctx.enter_context(tc.tile_pool(name="weights", bufs=1))
        spool = ctx.enter_context(tc.tile_pool(name="state", bufs=1))
        xpool = ctx.enter_context(tc.tile_pool(name="xin", bufs=4))
        vpool = ctx.enter_context(tc.tile_pool(name="vws", bufs=2))
        opool = ctx.enter_context(tc.tile_pool(name="outs", bufs=1))
        ppool = ctx.enter_context(tc.tile_pool(name="psum", bufs=1,
                                               space="PSUM"))

        x_stage = {}

        def stage_x(t):
            xhi_t = xpool.tile([128, NK * BL], F32R, name="xhi", tag="xhi")
            nc.sync.dma_start(out=xhi_t[:], in_=xhi_d.ap()[t])
            x8_t = None
            if NP:
                x8_t = xpool.tile([128, NP * NK * BL], F8E5, name="x8",
                                  tag="x8")
                nc.sync.dma_start(out=x8_t[:], in_=x8_d.ap()[t])
            x_stage[t] = (xhi_t, x8_t)

        # ---- load weights/biases in wave order, x[0]/x[1] interleaved ----
        stage_x(0)
        wr_t = [[None] * NK for _ in range(4)]
        wl_t = [[None] * NK for _ in range(4)]
        dr_t = {}
        b_t = [None] * 4

        def load_dr(name):
            tiles = []
            for p in range(2):
                tt = wpool.tile([128, 2 * H], F8E5, name=f"{name}{p}",
                                tag=f"{name}{p}")
                nc.sync.dma_start(out=tt[:], in_=dr_d[name].ap()[p])
                tiles.append(tt)
            dr_t[name] = tiles

        for i in range(4):
            fo = dims_out[i]
            for k in range(NK):
                if i == 3:
                    w3c = wpool.tile([128, 2 * A], F32R, name=f"w3c{k}",
                                     tag=f"w3c{k}")
                    nc.sync.dma_start(out=w3c[:, 0:A],
                                      in_=wr_d[i].ap()[k * 128:(k + 1) * 128, :])
                    nc.sync.dma_start(out=w3c[:, A:2 * A],
                                      in_=wl_d[3].ap()[k * 128:(k + 1) * 128, :])
                    wr_t[i][k] = w3c
                    continue
                wr_t[i][k] = wpool.tile([128, fo], F32R, name=f"w{i}r{k}",
                                        tag=f"w{i}r{k}")
                nc.sync.dma_start(out=wr_t[i][k][:],
                                  in_=wr_d[i].ap()[k * 128:(k + 1) * 128, :])
                if wl_d[i] is not None:
                    wl_t[i][k] = wpool.tile([128, fo], F32R, name=f"w{i}l{k}",
                                            tag=f"w{i}l{k}")
                    nc.sync.dma_start(out=wl_t[i][k][:],
                                      in_=wl_d[i].ap()[k * 128:(k + 1) * 128, :])
            # DR tiles for this layer
            if i == 0:
                if T2MODE.startswith("dr"):
                    load_dr("wh8a")
                    if T2MODE in ("dr2", "dr3"):
                        load_dr("wh8b")
                if T3MODE == "dr3":
                    load_dr("wl0a")
                    load_dr("wl0b")
            elif i in (1, 2) and LO12MODE.startswith("dr"):
                load_dr(f"wl{i}a")
                if LO12MODE == "dr2":
                    load_dr(f"wl{i}b")
            # biases
            nchunk = fo // 128 if fo >= 128 else 1
            if i == 3:
                b_t[i] = wpool.tile([2 * A, 1], F32, name="b3", tag="b3")
                nc.vector.memset(b_t[i][:], 0.0)
                nc.sync.dma_start(out=b_t[i][0:A, :], in_=b_d[i].ap()[0:A, :])
            else:
                b_t[i] = wpool.tile([128, nchunk], F32, name=f"b{i}",
                                    tag=f"b{i}")
                for m in range(nchunk):
                    pp = min(128, fo)
                    nc.sync.dma_start(out=b_t[i][0:pp, m:m + 1],
                                      in_=b_d[i].ap()[m * 128:m * 128 + pp, :])
            if i == 0:
                stage_x(1)

        warm_t = wpool.tile([128, 128], F32, name="warm", tag="warm")
        nc.vector.memset(warm_t[:], 0.0)

        # ---- persistent PSUM: c per layer ----
        c_ps = [
            ppool.tile([128, NM * BL], F32, name="c0", tag="c0"),
            ppool.tile([128, NM * BL], F32, name="c1", tag="c1"),
            ppool.tile([128, NM * BL], F32, name="c2", tag="c2"),
            ppool.tile([2 * A, BL], F32, name="c3", tag="c3"),
        ]
        out_acc = opool.tile([A, BL], F32, name="outacc", tag="outacc")
        nc.vector.memset(out_acc[:], 0.0)

        # persistent VS = v*(v<=vth) per layer (w34 = 0.75*VS)
        vs_t = [
            spool.tile([128, NM * BL], F32, name="vs0", tag="vs0"),
            spool.tile([128, NM * BL], F32, name="vs1", tag="vs1"),
            spool.tile([128, NM * BL], F32, name="vs2", tag="vs2"),
            spool.tile([A, BL], F32, name="vs3", tag="vs3"),
        ]
        for i in range(4):
            nc.vector.memset(vs_t[i][:], 0.0)

        Ident = mybir.ActivationFunctionType.Identity
        AOT = mybir.AluOpType
        s_tiles = [None] * 4

        def dr_rhs_x(x8_t, q, p):
            # piece q, k-tile pair p -> [128, 2, BL]
            v = x8_t[:].rearrange("a (q k n) -> a q k n", q=NP, k=NK)
            return v[:, q, 2 * p:2 * p + 2, :]

        def dr_rhs_s(s8_t, p):
            v = s8_t[:].rearrange("a (k n) -> a k n", k=NK)
            return v[:, 2 * p:2 * p + 2, :]

        def dr_lhs(name, p, m):
            return dr_t[name][p][:].rearrange("a (j m) -> a j m", j=2)[
                :, :, m * 128:(m + 1) * 128]

        def emit_layer(i, t, layer_r):
            fo = dims_out[i]
            nchunk = fo // 128 if fo >= 128 else 1
            pp = min(128, fo)
            dp = 2 * A if i == 3 else pp
            ps = c_ps[i]

            def emit_decay():
                for m in range(nchunk):
                    nc.scalar.activation(
                        ps[0:dp, m * BL:(m + 1) * BL],
                        ps[0:dp, m * BL:(m + 1) * BL],
                        Ident, bias=b_t[i][0:dp, m:m + 1], scale=CDECAY)

            if t > 0 and not EARLY_DECAY:
                emit_decay()

            # ---- matmul accumulation into PSUM ----
            for m in range(nchunk):
                out_ap = ps[0:dp, m * BL:(m + 1) * BL]
                first = (t == 0 and (m * BL) % 512 == 0)
                if i == 0:
                    xhi_t, x8_t = layer_r
                    piece_idx = {nm: qi for qi, nm in enumerate(x_pieces())}
                    # T1: W_hi @ x_hi (f32r)
                    for k in range(NK):
                        nc.tensor.matmul(
                            out_ap, wr_t[0][k][:, m * pp:(m + 1) * pp],
                            xhi_t[:, k * BL:(k + 1) * BL],
                            start=(first and k == 0), stop=True,
                            skip_group_check=True)
                    # T2: W_hi @ x_lo
                    if True:
                        for p in range(2):
                            nc.tensor.matmul(
                                out_ap, dr_lhs("wh8a", p, m),
                                dr_rhs_x(x8_t, piece_idx["xl8a"], p),
                                start=False, stop=True, perf_mode=DRMODE,
                                skip_group_check=True)
                        if T2MODE in ("dr2", "dr3"):
                            for p in range(2):
                                nc.tensor.matmul(
                                    out_ap, dr_lhs("wh8b", p, m),
                                    dr_rhs_x(x8_t, piece_idx["xl8a"], p),
                                    start=False, stop=True, perf_mode=DRMODE,
                                    skip_group_check=True)
                        if T2MODE == "dr3":
                            for p in range(2):
                                nc.tensor.matmul(
                                    out_ap, dr_lhs("wh8a", p, m),
                                    dr_rhs_x(x8_t, piece_idx["xl8b"], p),
                                    start=False, stop=True, perf_mode=DRMODE,
                                    skip_group_check=True)
                    # T3: W_lo @ x
                    if T3MODE == "f32r":
                        for k in range(NK):
                            nc.tensor.matmul(
                                out_ap, wl_t[0][k][:, m * pp:(m + 1) * pp],
                                xhi_t[:, k * BL:(k + 1) * BL],
                                start=False, stop=True, skip_group_check=True)
                    else:
                        qa, qb = piece_idx["x8a"], piece_idx["x8b"]
                        for p in range(2):
                            for lhs_nm, q in (("wl0a", qa), ("wl0b", qa),
                                              ("wl0a", qb)):
                                nc.tensor.matmul(
                                    out_ap, dr_lhs(lhs_nm, p, m),
                                    dr_rhs_x(x8_t, q, p),
                                    start=False, stop=True, perf_mode=DRMODE,
                                    skip_group_check=True)
                elif i == 3:
                    s_r, s8_r = layer_r
                    rhs_src = s8_r if SFP8 else s_r
                    for k in range(NK):
                        nc.tensor.matmul(
                            out_ap, wr_t[3][k][:, 0:2 * A],
                            rhs_src[:, k * BL:(k + 1) * BL],
                            start=(first and k == 0), stop=True,
                            skip_group_check=True)
                else:
                    s_r, s8_r = layer_r
                    hi_rhs = s8_r if SFP8 else s_r
                    for k in range(NK):
                        nc.tensor.matmul(
                            out_ap, wr_t[i][k][:, m * pp:(m + 1) * pp],
                            hi_rhs[:, k * BL:(k + 1) * BL],
                            start=(first and k == 0), stop=True,
                            skip_group_check=True)
                    if LO12MODE == "f32r":
                        for k in range(NK):
                            nc.tensor.matmul(
                                out_ap, wl_t[i][k][:, m * pp:(m + 1) * pp],
                                s_r[:, k * BL:(k + 1) * BL],
                                start=False, stop=True, skip_group_check=True)
                    else:
                        for p in range(2):
                            nc.tensor.matmul(
                                out_ap, dr_lhs(f"wl{i}a", p, m),
                                dr_rhs_s(s8_r, p),
                                start=False, stop=True, perf_mode=DRMODE,
                                skip_group_check=True)
                        if LO12MODE == "dr2":
                            for p in range(2):
                                nc.tensor.matmul(
                                    out_ap, dr_lhs(f"wl{i}b", p, m),
                                    dr_rhs_s(s8_r, p),
                                    start=False, stop=True, perf_mode=DRMODE,
                                    skip_group_check=True)

            if t == 0:
                for m in range(nchunk):
                    nc.scalar.activation(
                        ps[0:dp, m * BL:(m + 1) * BL],
                        ps[0:dp, m * BL:(m + 1) * BL],
                        Ident, bias=b_t[i][0:dp, m:m + 1], scale=1.0)

            # ---- elementwise chain ----
            # v = 0.75*VS_old + c
            v_t = vpool.tile([pp, nchunk * BL], F32, name=f"v{i}", tag=f"v{i}")
            nc.vector.scalar_tensor_tensor(
                v_t[:], vs_t[i][:], VDECAY, ps[0:pp, 0:nchunk * BL],
                AOT.mult, AOT.add)
            if i == 3:
                # fold in the lo-half current
                nc.vector.tensor_tensor(v_t[:], v_t[:], ps[A:2 * A, 0:BL],
                                        AOT.add)
            # spikes
            s_t = None
            s8_t = None
            if i == 3:
                s_t = vpool.tile([pp, nchunk * BL], F32, name="s3", tag="s3",
                                 bufs=3)
                nc.vector.tensor_scalar(s_t[:], v_t[:], VTH, None, AOT.is_gt)
            else:
                if SFP8:
                    s8_t = vpool.tile([pp, nchunk * BL], F8E5, name=f"s8_{i}",
                                      tag=f"s8_{i}", bufs=3)
                    nc.vector.tensor_scalar(s8_t[:], v_t[:], VTH, SSC,
                                            AOT.is_gt, AOT.mult)
                else:
                    s_t = vpool.tile([pp, nchunk * BL], F32R, name=f"s{i}",
                                     tag=f"s{i}", bufs=3)
                    nc.vector.tensor_scalar(s_t[:], v_t[:], VTH, None,
                                            AOT.is_gt)
                    if i in (0, 1) and LO12MODE.startswith("dr"):
                        # fp8 copy for the DR lo-terms, on ACT (it has slack;
                        # DVE carries the v/cmp/VS chain)
                        s8_t = vpool.tile([pp, nchunk * BL], F8E5,
                                          name=f"s8_{i}", tag=f"s8_{i}",
                                          bufs=3)
                        nc.scalar.activation(s8_t[:], s_t[:], Ident,
                                             bias=0.0, scale=SSC)
            if EARLY_DECAY and t + 1 < T_steps:
                emit_decay()
            # VS = (v <= vth) * v
            nc.vector.scalar_tensor_tensor(
                vs_t[i][:], v_t[:], VTH, v_t[:], AOT.is_le, AOT.mult)
            s_tiles[i] = (s_t, s8_t)

        def emit_warm(n, ps):
            for _ in range(n):
                nc.tensor.matmul(ps[0:64, 0:64], warm_t[:, 0:64],
                                 warm_t[:, 64:128], start=True, stop=True,
                                 skip_group_check=True)

        if WARM:
            emit_warm(30, c_ps[1])

        for w in range(T_steps + 4):
            if WARM and w >= T_steps:
                emit_warm(10, c_ps[0])
            if w >= 1 and w + 1 < T_steps:
                stage_x(w + 1)
            prev_s = list(s_tiles)
            for i in range(4):
                t = w - i
                if not (0 <= t < T_steps):
                    continue
                layer_r = x_stage.pop(t) if i == 0 else prev_s[i - 1]
                emit_layer(i, t, layer_r)
            t3 = w - 4
            if 0 <= t3 < T_steps:
                nc.vector.tensor_tensor(out_acc[:], out_acc[:],
                                        prev_s[3][0][:], AOT.add)

        nc.sync.dma_start(out=out_d.ap(), in_=out_acc[:])

    nc.compile()
    return nc


def make_in_maps(x: np.ndarray, Ws, bs, T_steps: int):
    """Shard x over batch; build f32r hi + e5m2 ladder pieces."""
    shared = {}
    pieces = x_pieces()
    NP = len(pieces)
    for i in range(4):
        wt = np.ascontiguousarray(Ws[i].T)  # [fin, fout]
        hi, lo = split_f32r(wt)
        ss = SS if i >= 1 else 1.0  # spike-side prescale (exact in f32r)
        shared[f"w{i}r"] = (hi * np.float32(ss)).astype(np.float32)
        shared[f"b{i}"] = np.ascontiguousarray(
            bs[i].reshape(-1, 1).astype(np.float32))
        if i == 0:
            if T3MODE == "f32r":
                shared["w0l"] = lo
            if T2MODE.startswith("dr"):
                a = q8(hi, T2WS)
                shared["wh8a"] = pack_dr(a)
                if T2MODE in ("dr2", "dr3"):
                    av = a.astype(np.float32) * np.float32(1.0 / T2WS)
                    shared["wh8b"] = pack_dr(q8(hi - av, T2WS))
            if T3MODE == "dr3":
                a = q8(lo, T3WS)
                shared["wl0a"] = pack_dr(a)
                av = a.astype(np.float32) * np.float32(1.0 / T3WS)
                shared["wl0b"] = pack_dr(q8(lo - av, T3WS))
        elif i in (1, 2):
            if LO12MODE == "f32r":
                shared[f"w{i}l"] = (lo * np.float32(ss)).astype(np.float32)
            else:
                # rhs carries s*2^-8 -> lhs ladder scale 2^8 (true W_lo)
                a = q8(lo, LOWS)
                shared[f"wl{i}a"] = pack_dr(a)
                if LO12MODE == "dr2":
                    av = a.astype(np.float32) * np.float32(1.0 / LOWS)
                    shared[f"wl{i}b"] = pack_dr(q8(lo - av, LOWS))
        else:
            shared["w3l"] = (lo * np.float32(ss)).astype(np.float32)

    in_maps = []
    for c in range(NCORES):
        xb = np.ascontiguousarray(
            x[c * BL:(c + 1) * BL, :, :T_steps].transpose(2, 1, 0))
        # [T, S, BL] -> k-major [T, 128, NK, BL]
        xb = np.ascontiguousarray(
            xb.reshape(T_steps, NK, 128, BL).transpose(0, 2, 1, 3))
        xhi = rne11(xb)
        m = dict(shared)
        m["xhi"] = xhi.reshape(T_steps, 128, NK * BL)
        if NP:
            xlo = (xb - xhi).astype(np.float32)
            pcs = {}
            if "xl8a" in pieces:
                a = q8(xlo, T2XS)
                pcs["xl8a"] = a
                if "xl8b" in pieces:
                    av = a.astype(np.float32) * np.float32(1.0 / T2XS)
                    pcs["xl8b"] = q8(xlo - av, T2XS)
            if "x8a" in pieces:
                a = q8(xb, T3XS)
                pcs["x8a"] = a
                av = a.astype(np.float32) * np.float32(1.0 / T3XS)
                pcs["x8b"] = q8(xb - av, T3XS)
            x8 = np.stack([pcs[nm] for nm in pieces], axis=1)  # [T,NP,128,NK,BL]
            m["x8"] = np.ascontiguousarray(
                x8.transpose(0, 2, 1, 3, 4).reshape(T_steps, 128,
                                                    NP * NK * BL))
        in_maps.append(m)
    return in_maps


_NC_CACHE = {}


def run(x, Ws, bs, T_steps=T, trace=False):
    if T_steps not in _NC_CACHE:
        _NC_CACHE[T_steps] = build_nc(T_steps)
    nc = _NC_CACHE[T_steps]
    in_maps = make_in_maps(x, Ws, bs, T_steps)
    res = bass_utils.run_bass_kernel_spmd(
        nc, in_maps, core_ids=list(range(NCORES)), trace=trace)
    outs = []
    for c in range(NCORES):
        o = res.results[c]["out"]
        outs.append(o.T)
    full = np.concatenate(outs, axis=0) / np.float32(T_steps)
    return full.astype(np.float32), res


def kernel(**inputs) -> np.ndarray:
    x = np.asarray(inputs["x"], dtype=np.float32)
    Ws = [np.asarray(inputs[f"W{i}"], dtype=np.float32) for i in range(4)]
    bs = [np.asarray(inputs[f"b{i}"], dtype=np.float32) for i in range(4)]
    out, _ = run(x, Ws, bs, T_steps=x.shape[2])
    return out


# revision 12
# speedup vs baseline: 1.1273x; 1.0460x over previous
"""Trainium2 Bass kernel for a 4-layer spiking actor network (SNN scan).

Reference computation (per timestep t of 50, batch B=2048):
    for layer i in 0..3:
        c_i = 0.5*c_i + in @ W_i.T + b_i
        v_i = 0.75*v_i*(1-s_i) + c_i
        s_i = (v_i > 0.5)
        in = s_i
    out = sum_t s_3 / 50

Strategy:
  - Data parallel over 8 NeuronCores: batch 2048 -> 256 per core; weights
    replicated. No cross-core communication.
  - Matmuls in float32r (11-bit mantissa, 1 cyc/col) with a hi/lo split:
    W = rne11(W) + rne11(W - rne11(W)); x likewise; spikes are 0/1 exact.
    Near-fp32 totals are required: the spiking dynamics amplify rounding
    into spike flips that the grader's relative-error gate counts.
  - L1/L2 W_lo correction terms run as SINGLE fp8 DoubleRow matmuls
    (measured 109 ns for a 256-deep-contraction instruction vs 2x121 ns
    for the two f32r k-tile matmuls they replace): lhsT = e4m3(W_lo*2^14),
    rhs = e5m2 spikes valued s*2^-14 (exact; 2^-14 is e5m2's min normal).
    e4m3's 2^-4 relative error on a 2^-12-scale correction costs ~5e-3
    output rel-L2 on this input set (gate 2e-2; measured in errmodel4.py).
    Multi-term fp8 ladders are NOT used: a DR instruction costs ~0.9x an
    f32r instruction, so >=2-term ladders have no advantage (measured).
  - Elementwise chain per layer (v-add / spike-compare / w34-mult on DVE,
    n75 and the c-decay on ACT) is the v1 structure: a fused
    scalar_tensor_tensor variant was tried and REVERTED — stt big-ops
    measure ~1.4us vs tensor_tensor's ~1.2us on DVE, which made DVE the
    wave pacer (+100us). The fp8 spike copies for the DR terms are cast on
    ACT. Engine totals at T=50: PE ~609us, DVE ~533us, ACT ~530us.
  - Membrane current c lives permanently in PSUM; tensor engine accumulates
    on top (start=False); ACT does in-place decay c <- 0.5c + b between
    steps, emitted right after the v-read so it lands early.
  - The (timestep, layer) grid is emitted as a layer-pipelined wavefront
    (wave w = layer i at t = w - i); dummy WARM matmuls hold the HAM clock
    gate at 2.4 GHz through startup and drain.
  - T2MODE=dr1e4 (layer-0 x_lo term as a single DR inst) measures 645us but
    doubles the error (rel_l2 1.01e-2, 2.1% elements differing) — rejected
    to keep margin under the 2e-2 gate.
  - Measured: v1 baseline 757us -> 665us, rel_l2 vs jax reference 5.4e-3
    (48 -> 525 flipped output elements of 65536; gate 2e-2).
"""
import os
import sys
import numpy as np
from contextlib import ExitStack

if os.path.isdir("/opt/trn_rl_repo"):
    sys.path.insert(0, "/opt/trn_rl_repo")

import ml_dtypes
import concourse.bass as bass
import concourse.tile as tile
from concourse import bacc, mybir
from concourse import bass_utils

F32 = mybir.dt.float32
F32R = mybir.dt.float32r
F8E5 = mybir.dt.float8e5
F8E4 = mybir.dt.float8e4
E5 = ml_dtypes.float8_e5m2
E4 = ml_dtypes.float8_e4m3
DRMODE = mybir.MatmulPerfMode.DoubleRow

B, S, T, H, A = 2048, 512, 50, 512, 32
NCORES = 8
BL = B // NCORES  # 256 batch per core
NK = 4            # k-tiles per 512-dim contraction
NM = 4
CDECAY, VDECAY, VTH = 0.5, 0.75, 0.5

# LO12MODE: L1/L2 W_lo @ s term: "f32r" (exact, 2 k-tile insts per pair),
# "dr1e4" (single e4m3xe5m2 DR inst per pair), "dr2" (two-term e5m2 ladder).
LO12MODE = os.environ.get("LO12MODE", "dr1e4")
# T2MODE: W_hi @ x_lo term of layer 0: "f32r" | "dr1e4"
T2MODE = os.environ.get("T2MODE", "f32r")
WARM = int(os.environ.get("WARM", "1"))
EARLY_DECAY = int(os.environ.get("EARLY_DECAY", "1"))

LOWS = 2.0**14   # L1/L2 dr1e4 lhsT scale; rhs spikes valued 2^-14
SSC = 2.0**-14
T2WS, T2XS = 2.0**-1, 2.0**1  # T2 dr1e4 scales


def rne11(x: np.ndarray) -> np.ndarray:
    """Round fp32 to 11 explicit mantissa bits, nearest-even (== HW float32r)."""
    u = np.ascontiguousarray(x, dtype=np.float32).view(np.uint32)
    lsb = (u >> np.uint32(12)) & np.uint32(1)
    u2 = (u + np.uint32(0x7FF) + lsb) & np.uint32(0xFFFFF000)
    return u2.view(np.float32)


def split_f32r(x: np.ndarray):
    hi = rne11(np.ascontiguousarray(x, dtype=np.float32))
    lo = rne11((x - hi).astype(np.float32))
    return hi, lo


def pack_dr(w8: np.ndarray) -> np.ndarray:
    """[512, fout] fp8 -> [2 pairs, 128, 2*fout] DoubleRow lhsT layout."""
    fin, fout = w8.shape
    assert fin == 512
    return np.ascontiguousarray(
        w8.reshape(2, 2, 128, fout).transpose(0, 2, 1, 3).reshape(2, 128, 2 * fout))


def build_nc(T_steps: int):
    nc = bacc.Bacc("TRN2", target_bir_lowering=False, debug=False,
                   num_devices=NCORES)

    dims_out = [H, H, H, A]

    # x ships packed per k-tile: block k = [x_hi_k | x_lo_k] (f32r), so T1
    # and T2 share one DMA and lhsT.  With T2MODE=dr1e4 the lo blocks are
    # instead a separate e5m2 tensor.
    if T2MODE == "f32r":
        xpk_d = nc.dram_tensor("xpk", [T_steps, 128, NK * 2 * BL], F32R,
                               kind="ExternalInput")
        xl8_d = None
    else:
        xpk_d = nc.dram_tensor("xpk", [T_steps, 128, NK * BL], F32R,
                               kind="ExternalInput")
        xl8_d = nc.dram_tensor("xl8", [T_steps, 128, NK * BL], F8E5,
                               kind="ExternalInput")

    wr_d, wl_d, b_d = [], [None] * 4, []
    for i in range(4):
        wr_d.append(nc.dram_tensor(f"w{i}r", [S, dims_out[i]], F32R,
                                   kind="ExternalInput"))
        b_d.append(nc.dram_tensor(f"b{i}", [dims_out[i], 1], F32,
                                  kind="ExternalInput"))
    wl_d[0] = nc.dram_tensor("w0l", [S, H], F32R, kind="ExternalInput")
    if LO12MODE == "f32r":
        for i in (1, 2):
            wl_d[i] = nc.dram_tensor(f"w{i}l", [S, H], F32R,
                                     kind="ExternalInput")
    wl_d[3] = nc.dram_tensor("w3l", [S, A], F32R, kind="ExternalInput")

    dr_d = {}

    def dr_tensor(name, dt):
        dr_d[name] = (nc.dram_tensor(name, [2, 128, 2 * H], dt,
                                     kind="ExternalInput"), dt)

    if T2MODE == "dr1e4":
        dr_tensor("wh8a", F8E4)
    if LO12MODE == "dr1e4":
        for i in (1, 2):
            dr_tensor(f"wl{i}a", F8E4)
    elif LO12MODE == "dr2":
        for i in (1, 2):
            dr_tensor(f"wl{i}a", F8E5)
            dr_tensor(f"wl{i}b", F8E5)

    out_d = nc.dram_tensor("out", [A, BL], F32, kind="ExternalOutput")

    with tile.TileContext(nc) as tc, ExitStack() as ctx:
        wpool = ctx.enter_context(tc.tile_pool(name="weights", bufs=1))
        spool = ctx.enter_context(tc.tile_pool(name="state", bufs=1))
        xpool = ctx.enter_context(tc.tile_pool(name="xin", bufs=4))
        vpool = ctx.enter_context(tc.tile_pool(name="vws", bufs=2))
        opool = ctx.enter_context(tc.tile_pool(name="outs", bufs=1))
        ppool = ctx.enter_context(tc.tile_pool(name="psum", bufs=1,
                                               space="PSUM"))

        XW = NK * 2 * BL if T2MODE == "f32r" else NK * BL
        x_stage = {}

        def stage_x(t):
            xpk_t = xpool.tile([128, XW], F32R, name="xpk", tag="xpk")
            nc.sync.dma_start(out=xpk_t[:], in_=xpk_d.ap()[t])
            xl8_t = None
            if xl8_d is not None:
                xl8_t = xpool.tile([128, NK * BL], F8E5, name="xl8",
                                   tag="xl8")
                nc.sync.dma_start(out=xl8_t[:], in_=xl8_d.ap()[t])
            x_stage[t] = (xpk_t, xl8_t)

        stage_x(0)
        wr_t = [[None] * NK for _ in range(4)]
        wl_t = [[None] * NK for _ in range(4)]
        dr_t = {}
        b_t = [None] * 4

        def load_dr(name):
            dram, dt = dr_d[name]
            tiles = []
            for p in range(2):
                tt = wpool.tile([128, 2 * H], dt, name=f"{name}{p}",
                                tag=f"{name}{p}")
                nc.sync.dma_start(out=tt[:], in_=dram.ap()[p])
                tiles.append(tt)
            dr_t[name] = tiles

        for i in range(4):
            fo = dims_out[i]
            for k in range(NK):
                if i == 3:
                    w3c = wpool.tile([128, 2 * A], F32R, name=f"w3c{k}",
                                     tag=f"w3c{k}")
                    nc.sync.dma_start(out=w3c[:, 0:A],
                                      in_=wr_d[i].ap()[k * 128:(k + 1) * 128, :])
                    nc.sync.dma_start(out=w3c[:, A:2 * A],
                                      in_=wl_d[3].ap()[k * 128:(k + 1) * 128, :])
                    wr_t[i][k] = w3c
                    continue
                wr_t[i][k] = wpool.tile([128, fo], F32R, name=f"w{i}r{k}",
                                        tag=f"w{i}r{k}")
                nc.sync.dma_start(out=wr_t[i][k][:],
                                  in_=wr_d[i].ap()[k * 128:(k + 1) * 128, :])
                if wl_d[i] is not None:
                    wl_t[i][k] = wpool.tile([128, fo], F32R, name=f"w{i}l{k}",
                                            tag=f"w{i}l{k}")
                    nc.sync.dma_start(out=wl_t[i][k][:],
                                      in_=wl_d[i].ap()[k * 128:(k + 1) * 128, :])
            if i == 0 and T2MODE == "dr1e4":
                load_dr("wh8a")
            if i in (1, 2) and LO12MODE == "dr1e4":
                load_dr(f"wl{i}a")
            elif i in (1, 2) and LO12MODE == "dr2":
                load_dr(f"wl{i}a")
                load_dr(f"wl{i}b")
            nchunk = fo // 128 if fo >= 128 else 1
            if i == 3:
                b_t[i] = wpool.tile([2 * A, 1], F32, name="b3", tag="b3")
                nc.vector.memset(b_t[i][:], 0.0)
                nc.sync.dma_start(out=b_t[i][0:A, :], in_=b_d[i].ap()[0:A, :])
            else:
                b_t[i] = wpool.tile([128, nchunk], F32, name=f"b{i}",
                                    tag=f"b{i}")
                for m in range(nchunk):
                    pp = min(128, fo)
                    nc.sync.dma_start(out=b_t[i][0:pp, m:m + 1],
                                      in_=b_d[i].ap()[m * 128:m * 128 + pp, :])
            if i == 0:
                stage_x(1)

        warm_t = wpool.tile([128, 128], F32, name="warm", tag="warm")
        nc.vector.memset(warm_t[:], 0.0)
        # per-partition 0.75 constant, bias operand for the n75 activation
        b75_t = wpool.tile([128, 1], F32, name="b75", tag="b75")
        nc.vector.memset(b75_t[:], VDECAY)

        c_ps = [
            ppool.tile([128, NM * BL], F32, name="c0", tag="c0"),
            ppool.tile([128, NM * BL], F32, name="c1", tag="c1"),
            ppool.tile([128, NM * BL], F32, name="c2", tag="c2"),
            ppool.tile([2 * A, BL], F32, name="c3", tag="c3"),
        ]
        out_acc = opool.tile([A, BL], F32, name="outacc", tag="outacc")
        nc.vector.memset(out_acc[:], 0.0)

        # persistent w34 = 0.75 * v * (1 - s) per layer
        w34_t = [
            spool.tile([128, NM * BL], F32, name="w34_0", tag="w34_0"),
            spool.tile([128, NM * BL], F32, name="w34_1", tag="w34_1"),
            spool.tile([128, NM * BL], F32, name="w34_2", tag="w34_2"),
            spool.tile([A, BL], F32, name="w34_3", tag="w34_3"),
        ]
        for i in range(4):
            nc.vector.memset(w34_t[i][:], 0.0)

        Ident = mybir.ActivationFunctionType.Identity
        AOT = mybir.AluOpType
        s_tiles = [None] * 4

        def dr_rhs(t2d, p):
            v = t2d[:].rearrange("a (k n) -> a k n", k=NK)
            return v[:, 2 * p:2 * p + 2, :]

        def dr_lhs(name, p, m):
            return dr_t[name][p][:].rearrange("a (j m) -> a j m", j=2)[
                :, :, m * 128:(m + 1) * 128]

        def emit_layer(i, t, layer_r):
            fo = dims_out[i]
            nchunk = fo // 128 if fo >= 128 else 1
            pp = min(128, fo)
            dp = 2 * A if i == 3 else pp
            ps = c_ps[i]

            def emit_decay():
                for m in range(nchunk):
                    nc.scalar.activation(
                        ps[0:dp, m * BL:(m + 1) * BL],
                        ps[0:dp, m * BL:(m + 1) * BL],
                        Ident, bias=b_t[i][0:dp, m:m + 1], scale=CDECAY)

            if t > 0 and not EARLY_DECAY:
                emit_decay()

            for m in range(nchunk):
                out_ap = ps[0:dp, m * BL:(m + 1) * BL]
                first = (t == 0 and (m * BL) % 512 == 0)
                if i == 0:
                    xpk_t, xl8_t = layer_r
                    for k in range(NK):
                        xo = 2 * k * BL if T2MODE == "f32r" else k * BL
                        # T1: W_hi @ x_hi
                        nc.tensor.matmul(
                            out_ap, wr_t[0][k][:, m * pp:(m + 1) * pp],
                            xpk_t[:, xo:xo + BL],
                            start=(first and k == 0), stop=True,
                            skip_group_check=True)
                        if T2MODE == "f32r":
                            # T2: W_hi @ x_lo (same lhsT, adjacent rhs block)
                            nc.tensor.matmul(
                                out_ap, wr_t[0][k][:, m * pp:(m + 1) * pp],
                                xpk_t[:, xo + BL:xo + 2 * BL],
                                start=False, stop=True, skip_group_check=True)
                        # T3: W_lo @ x_hi
                        nc.tensor.matmul(
                            out_ap, wl_t[0][k][:, m * pp:(m + 1) * pp],
                            xpk_t[:, xo:xo + BL],
                            start=False, stop=True, skip_group_check=True)
                    if T2MODE == "dr1e4":
                        for p in range(2):
                            nc.tensor.matmul(
                                out_ap, dr_lhs("wh8a", p, m),
                                dr_rhs(xl8_t, p),
                                start=False, stop=True, perf_mode=DRMODE,
                                skip_group_check=True)
                elif i == 3:
                    s_r, _ = layer_r
                    for k in range(NK):
                        nc.tensor.matmul(
                            out_ap, wr_t[3][k][:, 0:2 * A],
                            s_r[:, k * BL:(k + 1) * BL],
                            start=(first and k == 0), stop=True,
                            skip_group_check=True)
                else:
                    s_r, s8_r = layer_r
                    for k in range(NK):
                        nc.tensor.matmul(
                            out_ap, wr_t[i][k][:, m * pp:(m + 1) * pp],
                            s_r[:, k * BL:(k + 1) * BL],
                            start=(first and k == 0), stop=True,
                            skip_group_check=True)
                    if LO12MODE == "f32r":
                        for k in range(NK):
                            nc.tensor.matmul(
                                out_ap, wl_t[i][k][:, m * pp:(m + 1) * pp],
                                s_r[:, k * BL:(k + 1) * BL],
                                start=False, stop=True, skip_group_check=True)
                    else:
                        for p in range(2):
                            nc.tensor.matmul(
                                out_ap, dr_lhs(f"wl{i}a", p, m),
                                dr_rhs(s8_r, p),
                                start=False, stop=True, perf_mode=DRMODE,
                                skip_group_check=True)
                        if LO12MODE == "dr2":
                            for p in range(2):
                                nc.tensor.matmul(
                                    out_ap, dr_lhs(f"wl{i}b", p, m),
                                    dr_rhs(s8_r, p),
                                    start=False, stop=True, perf_mode=DRMODE,
                                    skip_group_check=True)

            if t == 0:
                for m in range(nchunk):
                    nc.scalar.activation(
                        ps[0:dp, m * BL:(m + 1) * BL],
                        ps[0:dp, m * BL:(m + 1) * BL],
                        Ident, bias=b_t[i][0:dp, m:m + 1], scale=1.0)

            # v = c + w34_old  (w34 = 0.75 * v_prev * not-spiked)
            v_t = vpool.tile([pp, nchunk * BL], F32, name=f"v{i}", tag=f"v{i}")
            nc.vector.tensor_tensor(v_t[:], ps[0:pp, 0:nchunk * BL],
                                    w34_t[i][:], AOT.add)
            if i == 3:
                nc.vector.tensor_tensor(v_t[:], v_t[:], ps[A:2 * A, 0:BL],
                                        AOT.add)
            # spikes
            s_t = vpool.tile([pp, nchunk * BL], F32R if i < 3 else F32,
                             name=f"s{i}", tag=f"s{i}", bufs=4)
            nc.vector.tensor_scalar(s_t[:], v_t[:], VTH, None, AOT.is_gt)
            s8_t = None
            if i in (0, 1) and LO12MODE != "f32r":
                # fp8 spike copy for the DR lo terms, cast on ACT
                s8_t = vpool.tile([pp, nchunk * BL], F8E5, name=f"s8_{i}",
                                  tag=f"s8_{i}", bufs=4)
                nc.scalar.activation(s8_t[:], s_t[:], Ident, bias=0.0,
                                     scale=SSC)
            if EARLY_DECAY and t + 1 < T_steps:
                emit_decay()
            # n75 = 0.75 * (1 - s) as a linear ACT function of s (exact 0/1)
            n75_t = vpool.tile([pp, nchunk * BL], F32, name=f"n{i}",
                               tag=f"n{i}")
            nc.scalar.activation(n75_t[:], s_t[:], Ident,
                                 bias=b75_t[0:pp, :], scale=-VDECAY)
            nc.vector.tensor_tensor(w34_t[i][:], v_t[:], n75_t[:], AOT.mult)
            s_tiles[i] = (s_t, s8_t)

        def emit_warm(n, ps):
            for _ in range(n):
                nc.tensor.matmul(ps[0:64, 0:64], warm_t[:, 0:64],
                                 warm_t[:, 64:128], start=True, stop=True,
                                 skip_group_check=True)

        if WARM:
            emit_warm(30, c_ps[1])

        for w in range(T_steps + 4):
            if WARM and w >= T_steps:
                emit_warm(10, c_ps[0])
            if w >= 1 and w + 1 < T_steps:
                stage_x(w + 1)
            prev_s = list(s_tiles)
            for i in range(4):
                t = w - i
                if not (0 <= t < T_steps):
                    continue
                layer_r = x_stage.pop(t) if i == 0 else prev_s[i - 1]
                emit_layer(i, t, layer_r)
            t3 = w - 4
            if 0 <= t3 < T_steps:
                nc.vector.tensor_tensor(out_acc[:], out_acc[:],
                                        prev_s[3][0][:], AOT.add)

        nc.sync.dma_start(out=out_d.ap(), in_=out_acc[:])

    nc.compile()
    return nc


def make_in_maps(x: np.ndarray, Ws, bs, T_steps: int):
    shared = {}
    for i in range(4):
        wt = np.ascontiguousarray(Ws[i].T)  # [fin, fout]
        hi, lo = split_f32r(wt)
        shared[f"w{i}r"] = hi
        shared[f"b{i}"] = np.ascontiguousarray(
            bs[i].reshape(-1, 1).astype(np.float32))
        if i == 0:
            shared["w0l"] = lo
            if T2MODE == "dr1e4":
                shared["wh8a"] = pack_dr(
                    (hi * np.float32(T2WS)).astype(E4))
        elif i in (1, 2):
            if LO12MODE == "f32r":
                shared[f"w{i}l"] = lo
            elif LO12MODE == "dr1e4":
                shared[f"wl{i}a"] = pack_dr(
                    (lo * np.float32(LOWS)).astype(E4))
            else:  # dr2: two-term e5m2 ladder at 2^8 (rhs then s*2^-8)
                a = (lo * np.float32(2.0**8)).astype(E5)
                av = a.astype(np.float32) * np.float32(2.0**-8)
                shared[f"wl{i}a"] = pack_dr(a)
                shared[f"wl{i}b"] = pack_dr(
                    ((lo - av) * np.float32(2.0**8)).astype(E5))
        else:
            shared["w3l"] = lo

    in_maps = []
    for c in range(NCORES):
        xb = np.ascontiguousarray(
            x[c * BL:(c + 1) * BL, :, :T_steps].transpose(2, 1, 0))
        xb = np.ascontiguousarray(
            xb.reshape(T_steps, NK, 128, BL).transpose(0, 2, 1, 3))
        xhi = rne11(xb)
        m = dict(shared)
        if T2MODE == "f32r":
            xlo = (xb - xhi).astype(np.float32)
            xcat = np.ascontiguousarray(
                np.stack([xhi, xlo], axis=3).reshape(T_steps, 128,
                                                     NK * 2 * BL))
            m["xpk"] = xcat
        else:
            m["xpk"] = np.ascontiguousarray(
                xhi.reshape(T_steps, 128, NK * BL))
            xlo = (xb - xhi).astype(np.float32)
            m["xl8"] = np.ascontiguousarray(
                (xlo * np.float32(T2XS)).astype(E5).reshape(
                    T_steps, 128, NK * BL))
        in_maps.append(m)
    return in_maps


_NC_CACHE = {}


def run(x, Ws, bs, T_steps=T, trace=False):
    if T_steps not in _NC_CACHE:
        _NC_CACHE[T_steps] = build_nc(T_steps)
    nc = _NC_CACHE[T_steps]
    in_maps = make_in_maps(x, Ws, bs, T_steps)
    res = bass_utils.run_bass_kernel_spmd(
        nc, in_maps, core_ids=list(range(NCORES)), trace=trace)
    outs = []
    for c in range(NCORES):
        o = res.results[c]["out"]
        outs.append(o.T)
    full = np.concatenate(outs, axis=0) / np.float32(T_steps)
    return full.astype(np.float32), res


def kernel(**inputs) -> np.ndarray:
    x = np.asarray(inputs["x"], dtype=np.float32)
    Ws = [np.asarray(inputs[f"W{i}"], dtype=np.float32) for i in range(4)]
    bs = [np.asarray(inputs[f"b{i}"], dtype=np.float32) for i in range(4)]
    out, _ = run(x, Ws, bs, T_steps=x.shape[2])
    return out
